# revision 1
# baseline (speedup 1.0000x reference)
"""Trainium2 Bass kernel for nn_MultiHeadGATLayerMerged.

Math (reference semantics):
  Wh[d,h] = x @ W[d,h]                                   (per batch b)
  e_src[d] = x @ (W[d,H-1] @ a[d,H-1,:OUT])              (only last head's
  e_dst[d] = x @ (W[d,H-1] @ a[d,H-1,OUT:])               logits survive)
  z_d[i,j] = leakyrelu(e_src[d][i] + e_dst[d][j], 0.01)
  e[i,j]   = z_{d*}[i,j],  d* = max d with A_d[i,j] != 0, else -inf
  P = exp(e); alpha = P / rowsum(P)
  out = (1/H) * diag(1/rowsum) * sum_d (A_d o P) @ (x @ (W[d,0]+W[d,1]))

Kernel strategy (one NeuronCore per batch element, 8 cores):
  Everything on the (N,N) attention plane is computed TRANSPOSED
  (j on partitions, i on free axis) so the masked matrices S_d^T feed the
  PE directly as lhsT — no large transposes anywhere.
  - priority merge: DVE copy_predicated with uint8 transposed masks
  - z builds: ACT Lrelu with per-partition bias (dst) over broadcast src rows
  - P = ACT Exp -> fp16; S_d = mask_fp16 * P (DVE 2x mode)
  - projections x@Wsum in fp32r (full PE speed, ~1e-4 accuracy)
  - main matmuls in fp16; rowsum via PE ones-column matmuls
"""

import numpy as np
import ml_dtypes
from contextlib import ExitStack

import concourse.bass as bass
import concourse.mybir as mybir
import concourse.tile as tile
from concourse.bass_utils import run_bass_kernel_spmd

dt = mybir.dt
AF = mybir.ActivationFunctionType

B, N, F, OUT, H, D = 8, 1024, 256, 256, 2, 4
P = 128
NJC = N // P   # j chunks (partition chunks of transposed plane)
NIC = N // P   # i chunks
FC = F // P    # f chunks for contraction


# ---------------------------------------------------------------------------
# BIR post-pass: several walrus instruction structs in this toolchain have
# very few semaphore-wait slots (CopyPredicated: 0, Matmult: ~1, ...).
# Hoist waits into standalone EventSemaphore instructions placed immediately
# before the owning instruction (same engine, program order => semantics
# identical: waits are monotone thresholds).
_ZERO_WAIT_TYPES = (mybir.InstCopyPredicated,)
_DEFAULT_LIMIT = 1


def _hoist_waits(nc):
    n_fixed = 0
    for fn in nc.m.functions:
        for bb in fn.blocks:
            insts = list(bb.instructions)
            new_insts = []
            for inst in insts:
                si = getattr(inst, "sync_info", None)
                if si is not None and si.on_wait:
                    limit = 0 if isinstance(inst, _ZERO_WAIT_TYPES) else _DEFAULT_LIMIT
                    waits = list(si.on_wait)
                    if len(waits) > limit:
                        excess = waits[: len(waits) - limit] if limit else waits
                        keep = waits[len(waits) - limit:] if limit else []
                        for k in range(0, len(excess), 2):
                            ev = mybir.InstEventSemaphore(
                                name=f"{inst.name}-hw{k}", ins=[], outs=[])
                            ev.engine = inst.engine
                            ev.debug = inst.debug
                            ev.sync_info = mybir.SyncInfo(
                                on_wait=excess[k:k + 2], on_update=[])
                            new_insts.append(ev)
                        inst.sync_info = mybir.SyncInfo(
                            on_wait=keep, on_update=list(si.on_update))
                        n_fixed += 1
                new_insts.append(inst)
            bb.instructions = new_insts
    return n_fixed


def _dedupe_ldweights(nc):
    """Drop LDWEIGHTS identical to the immediately preceding LDWEIGHTS on PE
    (stationary operand persists across matmuls; --enable-ldw-opt=false keeps
    walrus from doing this)."""
    n_drop = 0
    for fn in nc.m.functions:
        for bb in fn.blocks:
            insts = list(bb.instructions)
            new_insts = []
            prev_key = None
            for inst in insts:
                if isinstance(inst, mybir.InstLdweights):
                    key = repr(inst.ins[0])
                    si = getattr(inst, "sync_info", None)
                    clean = si is None or (not si.on_wait and not si.on_update)
                    if key == prev_key and clean:
                        n_drop += 1
                        continue
                    prev_key = key
                elif isinstance(inst, (mybir.InstMatmult, mybir.InstEventSemaphore)):
                    pass
                else:
                    if getattr(inst, "engine", None) == mybir.EngineType.PE:
                        prev_key = None
                new_insts.append(inst)
            bb.instructions = new_insts
    return n_drop


def _build(nc: bass.Bass):
    xT = nc.dram_tensor("xT", [F, N], dt.float32, kind="ExternalInput")
    mF16 = nc.dram_tensor("mF16", [D, N, N], dt.float16, kind="ExternalInput")
    wvec = nc.dram_tensor("wvec", [F, 2 * D], dt.float32, kind="ExternalInput")
    ws = nc.dram_tensor("ws", [D, F, OUT], dt.float32, kind="ExternalInput")
    out = nc.dram_tensor("out", [N, OUT], dt.float32, kind="ExternalOutput")

    with tile.TileContext(nc) as tc, ExitStack() as ctx:
        cpool = ctx.enter_context(tc.tile_pool(name="consts", bufs=1))
        spool = ctx.enter_context(tc.tile_pool(name="statics", bufs=1))

        onesf = cpool.tile([1, P], dt.float32, name="onesf", tag="onesf")
        nc.vector.memset(onesf[:], 1.0)
        onesfr = cpool.tile([1, P], dt.float32r, name="onesfr", tag="onesfr")
        nc.vector.tensor_copy(onesfr[:], onesf[:])
        onescol16 = cpool.tile([P, 1], dt.float16, name="onescol16", tag="onescol16")
        nc.vector.memset(onescol16[:], 1.0)

        # ---- load x^T (f on partitions), make fp32r copy for projections
        xt = []
        xtr = []
        for fc in range(FC):
            t = spool.tile([P, N], dt.float32, name=f"xt{fc}", tag=f"xt{fc}")
            nc.sync.dma_start(t[:], xT[fc * P:(fc + 1) * P, :])
            xt.append(t)
            tr = spool.tile([P, N], dt.float32r, name=f"xtr{fc}", tag=f"xtr{fc}")
            nc.vector.tensor_copy(tr[:], t[:])
            xtr.append(tr)

        # ---- load wvec chunks
        wv = []
        for fc in range(FC):
            t = spool.tile([P, 2 * D], dt.float32, name=f"wv{fc}", tag=f"wv{fc}")
            nc.sync.dma_start(t[:], wvec[fc * P:(fc + 1) * P, :])
            wv.append(t)

        # ---- load Wsum, round to fp32r
        wsr = [[None] * FC for _ in range(D)]
        for d in range(D):
            for fc in range(FC):
                t = spool.tile([P, OUT], dt.float32, name=f"ws{d}{fc}", tag=f"ws{d}{fc}")
                nc.sync.dma_start(t[:], ws[d, fc * P:(fc + 1) * P, :])
                tr = spool.tile([P, OUT], dt.float32r, name=f"wsr{d}{fc}", tag=f"wsr{d}{fc}")
                nc.vector.tensor_copy(tr[:], t[:])
                wsr[d][fc] = tr

        # ---- src rows (1, N) f32 per direction (M=1 matmuls, partition 0)
        # fp32r (1 cyc/row at N>=256) -- logits only see ~1e-4 rounding.
        wvr = []
        for fc in range(FC):
            t = spool.tile([P, 2 * D], dt.float32r, name=f"wvr{fc}", tag=f"wvr{fc}")
            nc.vector.tensor_copy(t[:], wv[fc][:])
            wvr.append(t)
        src_row = []
        with tc.tile_pool(name="srcps", bufs=2, space="PSUM") as srcps:
            for d in range(D):
                ps = srcps.tile([1, N], dt.float32, name=f"sps{d}", tag="sps")
                for hhalf in range(2):
                    sl = slice(hhalf * 512, (hhalf + 1) * 512)
                    for fc in range(FC):
                        nc.tensor.matmul(
                            ps[:, sl], wvr[fc][:, d:d + 1], xtr[fc][:, sl],
                            start=(fc == 0), stop=(fc == FC - 1))
                t = spool.tile([1, N], dt.float32, name=f"srcrow{d}", tag=f"srcrow{d}")
                nc.scalar.copy(t[:], ps[:])
                src_row.append(t)

        # ---- dst columns (128, D) f32 per j-chunk
        dst_col = []
        with tc.tile_pool(name="dstps", bufs=2, space="PSUM") as dstps:
            for jc in range(NJC):
                ps = dstps.tile([P, D], dt.float32, name=f"dps{jc}", tag="dps")
                for fc in range(FC):
                    nc.tensor.matmul(
                        ps[:], xtr[fc][:, jc * P:(jc + 1) * P], wvr[fc][:, D:2 * D],
                        start=(fc == 0), stop=(fc == FC - 1))
                t = spool.tile([P, D], dt.float32, name=f"dstcol{jc}", tag=f"dstcol{jc}")
                nc.scalar.copy(t[:], ps[:])
                dst_col.append(t)

        # ---- srcB_d: (128, N) broadcast of src_row[d] down partitions (fp32r)
        srcb = []
        with tc.tile_pool(name="bcps", bufs=2, space="PSUM") as bcps:
            for d in range(D):
                sr = spool.tile([1, N], dt.float32r, name=f"srcr{d}", tag=f"srcr{d}")
                nc.vector.tensor_copy(sr[:], src_row[d][:])
                ps = bcps.tile([P, N], dt.float32, name=f"bps{d}", tag="bps")
                for hhalf in range(2):
                    sl = slice(hhalf * 512, (hhalf + 1) * 512)
                    nc.tensor.matmul(ps[:, sl], onesfr[:], sr[:, sl],
                                     start=True, stop=True)
                t = spool.tile([P, N], dt.float16, name=f"srcb{d}", tag=f"srcb{d}")
                nc.scalar.copy(t[:], ps[:])
                srcb.append(t)

        # ---- persistent PSUM: output accumulators + rowsum
        outps_pool = ctx.enter_context(tc.tile_pool(name="outps", bufs=1, space="PSUM"))
        out_ps = [outps_pool.tile([P, 512], dt.float32, name=f"ops{q}", tag=f"ops{q}")
                  for q in range(4)]
        rs_ps = outps_pool.tile([1, N], dt.float32, name="rsps", tag="rsps")

        # ---- streaming pools for the main loop (chunk pairs in flight)
        loopctx = ctx.enter_context(ExitStack())
        mpool = loopctx.enter_context(tc.tile_pool(name="masks", bufs=4))
        zpool = loopctx.enter_context(tc.tile_pool(name="zs", bufs=4))
        epool = loopctx.enter_context(tc.tile_pool(name="es", bufs=4))
        ppool = loopctx.enter_context(tc.tile_pool(name="ps16", bufs=4))
        spool2 = loopctx.enter_context(tc.tile_pool(name="ss", bufs=3))
        wpool = loopctx.enter_context(tc.tile_pool(name="whsp", bufs=3))
        projps = loopctx.enter_context(tc.tile_pool(name="projps", bufs=2, space="PSUM"))


        # chunks processed in groups of 4 so ACT batches same-function ops
        # (each ACT function switch costs a ~1.3us table load on the
        # pt-critical path)
        for grp in range(NJC // 4):
            jcs = tuple(range(4 * grp, 4 * grp + 4))
            mu = {}
            mf = {}
            whs = {}
            zs = {}
            pt = {}
            for jc in jcs:
                jsl = slice(jc * P, (jc + 1) * P)
                for d in range(D):
                    t16 = mpool.tile([P, N], dt.float16, name=f"mf{d}", tag=f"mf{d}")
                    nc.sync.dma_start(t16[:], mF16[d, jsl, :])
                    mf[(jc, d)] = t16
                    # fp16 0/1 bitpattern doubles as a nonzero integer mask
                    mu[(jc, d)] = t16[:].bitcast(dt.uint16)

            # projections for this pair's chunks (keeps PE warm between
            # the attention matmul bursts)
            for jc in jcs:
                for d0 in (0, 2):
                    pss = [projps.tile([P, OUT], dt.float32, name=f"pps{d0+k}",
                                       tag="pps") for k in range(2)]
                    for fc in range(FC):
                        for k in range(2):
                            nc.tensor.matmul(
                                pss[k][:], xtr[fc][:, jc * P:(jc + 1) * P],
                                wsr[d0 + k][fc][:],
                                start=(fc == 0), stop=(fc == FC - 1),
                                skip_group_check=True)
                    for k in range(2):
                        t = wpool.tile([P, OUT], dt.float16, name=f"whs{d0+k}",
                                       tag=f"whs{d0+k}")
                        nc.scalar.copy(t[:], pss[k][:])
                        whs[(jc, d0 + k)] = t

            # z_d = src_d[i] + dst_d[j] on DVE (fp16 tensor_scalar 4x mode;
            # keeps the merge self-feeding on one engine). leakyrelu is
            # monotone so it is applied once AFTER the priority merge.
            for jc in jcs:
                for d in range(D):
                    z = zpool.tile([P, N], dt.float16, name=f"z{d}", tag=f"z{d}")
                    nc.vector.tensor_scalar(z[:], srcb[d][:],
                                            dst_col[jc][:, d:d + 1], None,
                                            op0=mybir.AluOpType.add)
                    zs[(jc, d)] = z

            # priority merge (d=3 wins), default -3000 (post-lrelu -30)
            es = {}
            for jc in jcs:
                e = epool.tile([P, N], dt.float16, name="e", tag="e")
                nc.gpsimd.memset(e[:], -3000.0)
                for d in range(D):
                    nc.vector.copy_predicated(e[:], mu[(jc, d)], zs[(jc, d)][:])
                es[jc] = e
            # batched per pair to limit ACT LUT swaps: Lrelu x2 then Exp x2
            elr = {}
            for jc in jcs:
                t = ppool.tile([P, N], dt.float16, name="elr", tag="elr")
                nc.scalar.activation(t[:], es[jc][:], AF.Lrelu, bias=0.0,
                                     scale=1.0, alpha=0.01)
                elr[jc] = t
            for jc in jcs:
                p16 = ppool.tile([P, N], dt.float16, name="pt", tag="pt")
                nc.scalar.activation(p16[:], elr[jc][:], AF.Exp, bias=0.0,
                                     scale=1.0)
                pt[jc] = p16

            for jc in jcs:
                # rowsum row: rs[0, i] += sum_j P^T[j, i]; ones column is the
                # stationary operand so consecutive LDWEIGHTS dedupe.
                # (start=True per bank-half clears only that bank's state.)
                for ih in range(2):
                    isl = slice(ih * 512, (ih + 1) * 512)
                    nc.tensor.matmul(rs_ps[:, isl], onescol16[:], pt[jc][:, isl],
                                     start=(jc == 0),
                                     stop=(jc == NJC - 1),
                                     skip_group_check=True)

                # S_d = mask * P (fp16), then main matmuls
                for d in range(D):
                    s = spool2.tile([P, N], dt.float16, name=f"s{d}", tag=f"s{d}")
                    nc.vector.tensor_mul(s[:], mf[(jc, d)][:], pt[jc][:])
                    for ic in range(NIC):
                        q, r = divmod(ic, 2)
                        nc.tensor.matmul(
                            out_ps[q][:, r * 256:(r + 1) * 256],
                            s[:, ic * P:(ic + 1) * P], whs[(jc, d)][:],
                            start=(jc == 0 and d == 0 and r == 0),
                            stop=(jc == NJC - 1 and d == D - 1 and r == 1),
                            skip_group_check=True)

        # ---- epilogue: out[i, :] *= 1 / (H * rowsum[i])
        loopctx.close()

        rs_row = spool.tile([1, N], dt.float32, name="rsrow", tag="rsrow")
        nc.vector.tensor_copy(rs_row[:], rs_ps[:])
        inv_row = spool.tile([1, N], dt.float32, name="invrow", tag="invrow")
        nc.vector.reciprocal(inv_row[:], rs_row[:])
        nc.vector.tensor_scalar_mul(inv_row[:], inv_row[:], 1.0 / H)
        ones11 = cpool.tile([1, 1], dt.float32, name="ones11", tag="ones11")
        nc.vector.memset(ones11[:], 1.0)
        inv = spool.tile([P, NIC], dt.float32, name="inv", tag="inv")
        with tc.tile_pool(name="invps", bufs=2, space="PSUM") as invps:
            for ic in range(NIC):
                ps = invps.tile([P, 1], dt.float32, name=f"ivp{ic}", tag="ivp")
                nc.tensor.matmul(ps[:], inv_row[:, ic * P:(ic + 1) * P], ones11[:],
                                 start=True, stop=True)
                nc.vector.tensor_copy(inv[:, ic:ic + 1], ps[:])
        for ic in range(NIC):
            q, r = divmod(ic, 2)
            o = spool.tile([P, OUT], dt.float32, name=f"osb{ic}", tag=f"osb{ic}")
            nc.vector.tensor_scalar(o[:], out_ps[q][:, r * 256:(r + 1) * 256],
                                    inv[:, ic:ic + 1], None,
                                    op0=mybir.AluOpType.mult)
            nc.sync.dma_start(out[ic * P:(ic + 1) * P, :], o[:])

    return nc


_CACHED = {}


def _get_nc():
    if "nc" not in _CACHED:
        nc = bass.Bass()
        _build(nc)
        _hoist_waits(nc)
        _dedupe_ldweights(nc)
        _CACHED["nc"] = nc
    return _CACHED["nc"]


def kernel(x, A_U, A_D, A_R, A_L, W, a):
    x = np.asarray(x, dtype=np.float32)
    W = np.asarray(W, dtype=np.float32)
    a = np.asarray(a, dtype=np.float32)

    masks = [np.asarray(m) for m in (A_U, A_D, A_R, A_L)]
    # transposed masks (j on rows): mT[d][j, i] = A_d[i, j]
    mT = np.stack([np.ascontiguousarray(m.T) for m in masks])
    m_f16 = (mT != 0).astype(np.float16)

    # attention vector folding (last head only survives the merge)
    wv_cols = [W[d, H - 1] @ a[d, H - 1, :OUT] for d in range(D)] + \
              [W[d, H - 1] @ a[d, H - 1, OUT:] for d in range(D)]
    wvec = np.stack(wv_cols, axis=1).astype(np.float32)   # (F, 2D)
    ws = np.ascontiguousarray(W.sum(axis=1), dtype=np.float32)  # (D, F, OUT)

    nc = _get_nc()
    core_ids = list(range(B))
    in_maps = []
    for b in range(B):
        in_maps.append({
            "xT": np.ascontiguousarray(x[b].T),
            "mF16": m_f16,
            "wvec": wvec,
            "ws": ws,
        })
    res = run_bass_kernel_spmd(nc, in_maps, core_ids)
    out = np.stack([res.results[b]["out"] for b in range(B)], axis=0)
    return out.astype(np.float32)

